# revision 37
# baseline (speedup 1.0000x reference)
"""Trainium2 Bass kernel for GQA attention with RoPE (nn_Attention_21603685499660).

Shapes (hardcoded): x [2, 2048, 4096], H=32 Q heads, KVH=8 KV heads, HD=128.
Sharding over 8 NeuronCores: core c -> batch b = c//4, head-group g = c%4
(8 Q heads, 2 KV heads per core).  Each core computes a partial output
(its heads' attention output through its slice of wo); the host sums the
4 partials per batch.  No on-device collectives.

v2 (causal path) — software-pipelined window schedule that keeps TensorE
dense end-to-end:
  window 0:   QKV projection chunk 0 (+ PE warmup through the HAM window)
  window 1-3: QKV projection chunk w interleaved with attention chunk w-1
              (attention tiles are pumped between projection matmul groups,
              so the ACT-paced attention never idles the PE and projection
              DMA waits are covered by attention work)
  window 4:   attention chunk 3 with output-projection (po) filler for
              chunks 0-2, then po chunk 3 drain.
Other changes vs v1:
  - softmax 1/l via one batched DVE reciprocal_approx_fast per chunk on an
    [8,512] PSUM accumulator (per-head selector matmuls route each head's
    denominator to its own partition row).  This removes the per-head
    Ln/Exp pair and with it 64 ACT_TABLE_LOAD switches (~83us ScalarE).
  - causal diagonal blocks computed at column granularity: QK/exp/PV run
    on [qoff:] only; the in-tile staircase uses one resident [128,128]
    triangle mask (mask DMA drops from 2MB to 32KB).
"""

from contextlib import ExitStack

import numpy as np
import ml_dtypes

import concourse.bass as bass
import concourse.tile as tile
from concourse import bacc, mybir
from concourse.bass_utils import run_bass_kernel_spmd

B, S, D = 2, 2048, 4096
H, KVH, HD = 32, 8, 128
N_CORES = 8
GROUPS = 4            # head groups (tensor-parallel dim); B * GROUPS = 8 cores
HL = H // GROUPS      # 8 local Q heads
KVL = KVH // GROUPS   # 2 local KV heads
FQK = HL + KVL        # 10 feature tiles of 128 (Q heads then K heads)
NJ = S // 512         # 4 seq chunks of 512
NT = S // 128         # 16 seq tiles of 128
ND = D // 128         # 32 contraction tiles
BF = mybir.dt.bfloat16
F32 = mybir.dt.float32

_BUILD_CACHE: dict = {}

_EXH = object()  # generator-exhausted sentinel


def _build_causal():
    nc = bacc.Bacc("TRN2", target_bir_lowering=False, debug=False,
                   num_devices=N_CORES)

    xt_d = nc.dram_tensor("xt", [128, ND, S], BF, kind="ExternalInput").ap()
    wqk_d = nc.dram_tensor("wqk", [FQK, 128, ND, 128], BF, kind="ExternalInput").ap()
    wv_d = nc.dram_tensor("wv", [128, ND, KVL * HD], BF, kind="ExternalInput").ap()
    wo_d = nc.dram_tensor("wo", [128, HL, D], BF, kind="ExternalInput").ap()
    cos_d = nc.dram_tensor("cosd", [128, S], BF, kind="ExternalInput").ap()
    sin_d = nc.dram_tensor("sind", [128, S], BF, kind="ExternalInput").ap()
    tri_d = nc.dram_tensor("trimask", [128, 128], BF, kind="ExternalInput").ap()
    selL_d = nc.dram_tensor("sell", [128, HL * 8], BF, kind="ExternalInput").ap()
    selB_d = nc.dram_tensor("selb", [8, HL * 128], BF, kind="ExternalInput").ap()
    po_d = nc.dram_tensor("po", [S, D], BF, kind="ExternalOutput").ap()

    with tile.TileContext(nc) as tc, ExitStack() as ctx:
        resident = ctx.enter_context(tc.tile_pool(name="resident", bufs=1))
        qkv = ctx.enter_context(tc.tile_pool(name="qkv", bufs=1))
        # attention-side pools, alive for the whole kernel
        ppool = ctx.enter_context(tc.tile_pool(name="ppool", bufs=6))
        qpool = ctx.enter_context(tc.tile_pool(name="qpool", bufs=2))
        npool = ctx.enter_context(tc.tile_pool(name="npool", bufs=2))
        ps_st = ctx.enter_context(tc.tile_pool(name="ps_st", bufs=3, space="PSUM"))
        ps_o = ctx.enter_context(tc.tile_pool(name="ps_o", bufs=2, space="PSUM"))
        ps_l = ctx.enter_context(tc.tile_pool(name="ps_l", bufs=1, space="PSUM"))

        ones_col = resident.tile([128, 1], BF)
        nc.vector.memset(ones_col[:], 1.0)
        zeros = resident.tile([128, 512], BF)
        nc.vector.memset(zeros[:], 0.0)
        trib = resident.tile([128, 128], BF)
        nc.sync.dma_start(out=trib[:], in_=tri_d[:])
        # selL[:, h*8:(h+1)*8]: lhsT routing head h's column-sum into
        # partition row h of the shared [8,512] l accumulator.
        selL = resident.tile([128, HL * 8], BF)
        nc.sync.dma_start(out=selL[:], in_=selL_d[:])
        # selB[:, h*128:(h+1)*128]: lhsT broadcasting linv row h to all 128
        # output partitions.
        selB = resident.tile([8, HL * 128], BF)
        nc.sync.dma_start(out=selB[:], in_=selB_d[:])

        # QTA: Q in [HD, head, seq] layout; each chunk's region is
        # overwritten in place with that head's (raw, then normalized)
        # attention output once its last QK read retires.
        QTA = qkv.tile([128, HL, S], BF)
        KT = qkv.tile([128, KVL, S], BF)
        V = qkv.tile([128, NT, KVL * HD], BF)  # [seq%128, seqtile, kv-head*HD]

        pending_po = []  # (qt, nn) output-projection groups ready to emit

        # ---- attention chunk generator (yields after each schedulable unit) --
        def attn_chunk(j):
            js = bass.ts(j, 512)
            nkt = 4 * (j + 1)
            l8 = ps_l.tile([8, 512], F32, tag="l8")
            for h in range(HL):
                hk = h // (HL // KVL)
                outp = ps_o.tile([128, 512], F32, tag="out")
                pts = []
                qoffs = []

                def emit_pv(t):
                    qo = qoffs[t]
                    nc.tensor.matmul(outp[:, qo:], V[:, t, bass.ts(hk, 128)],
                                     pts[t][:, qo:],
                                     start=(t == 0), stop=(t == nkt - 1))

                for t in range(nkt):
                    i = t - 4 * j  # diagonal sub-index (>=0 on the diagonal)
                    qo = 128 * i if i > 0 else 0
                    qoffs.append(qo)
                    stp = ps_st.tile([128, 512], F32, tag="st")
                    nc.tensor.matmul(
                        stp[:, qo:], KT[:, hk, bass.ts(t, 128)],
                        QTA[:, h, bass.DynSlice(j * 512 + qo, 512 - qo)],
                        start=True, stop=True)
                    pt = ppool.tile([128, 512], BF, tag="pt")
                    if qo:
                        nc.vector.memset(pt[:, :qo], 0.0)
                    nc.scalar.activation(out=pt[:, qo:], in_=stp[:, qo:],
                                         func=mybir.ActivationFunctionType.Exp)
                    if i >= 0:
                        # in-tile causal staircase: only the 128-wide band
                        # [qo, qo+128) is partially masked
                        nc.vector.tensor_mul(pt[:, qo:qo + 128],
                                             pt[:, qo:qo + 128], trib[:])
                    pts.append(pt)
                    if t > 0:
                        emit_pv(t - 1)
                    if t % 4 == 3:
                        a, b, c, dq = pts[-4:]
                        s1 = qpool.tile([128, 512], BF, tag="s1")
                        nc.vector.tensor_add(s1[:], a[:], b[:])
                        s2 = qpool.tile([128, 512], BF, tag="s2")
                        nc.vector.tensor_add(s2[:], c[:], dq[:])
                        qd = qpool.tile([128, 512], BF, tag="qd")
                        nc.vector.tensor_add(qd[:], s1[:], s2[:])
                        nc.tensor.matmul(l8[:], selL[:, bass.ts(h, 8)], qd[:],
                                         start=(h == 0 and t == 3),
                                         stop=(h == HL - 1 and t == nkt - 1))
                    yield
                emit_pv(nkt - 1)
                # raw (unnormalized) head output overwrites the dead Q region
                nc.vector.tensor_copy(QTA[:, h, js], outp[:])
                yield
            # chunk epilogue: one batched reciprocal for all 8 heads
            linv = npool.tile([8, 512], F32, tag="linv")
            nc.vector.reciprocal_approx_fast(out=linv[:], in_=l8[:])
            linvb = npool.tile([8, 512], BF, tag="linvb")
            nc.vector.tensor_copy(linvb[:], linv[:])
            yield
            for h in range(HL):
                rbp = ps_st.tile([128, 512], F32, tag="st")
                nc.tensor.matmul(rbp[:], selB[:, bass.ts(h, 128)], linvb[:],
                                 start=True, stop=True)
                nc.vector.tensor_mul(QTA[:, h, js], QTA[:, h, js], rbp[:])
                yield
            # nn-major so the first groups only need the preloaded wob1 half
            pending_po.extend(
                (qt, nn) for nn in range(D // 512)
                for qt in range(4 * j, 4 * j + 4))

        def pump(gen, k):
            if gen is None:
                return
            for _ in range(k):
                if next(gen, _EXH) is _EXH:
                    return

        # ---- windows 0-3: projection chunk w (+ attention chunk w-1) ----
        with tc.tile_pool(name="s1const", bufs=1) as s1const, \
             tc.tile_pool(name="xpool", bufs=2) as xpool, \
             tc.tile_pool(name="wpool", bufs=3) as wpool, \
             tc.tile_pool(name="tpool", bufs=3) as tpool, \
             tc.tile_pool(name="ps_qk", bufs=2, space="PSUM") as ps_qk:
            cosb = s1const.tile([128, S], BF)
            sinb = s1const.tile([128, S], BF)  # sign-folded: rows 0-63 = -sin
            wvb = s1const.tile([128, ND, KVL * HD], BF)
            # PE warm-up through the HAM window while the first DMAs land
            # (zeros rhs is memset locally, so no DMA gates the first matmul)
            for _ in range(22):
                wt = ps_qk.tile([128, 512], F32, tag="qk")
                nc.tensor.matmul(wt[:1, :], ones_col[:], zeros[:],
                                 start=True, stop=True)

            def rope_emit(ent):
                # rotate-half via SBUF->SBUF DMA (partition swap); the sign
                # of the rotated half is folded into sinb host-side
                raw, f, js = ent
                rot = tpool.tile([128, 512], BF, tag="rot")
                nc.sync.dma_start(out=rot[:64, :], in_=raw[64:, :])
                nc.sync.dma_start(out=rot[64:, :], in_=raw[:64, :])
                t1 = tpool.tile([128, 512], F32, tag="t1")
                nc.vector.tensor_mul(t1[:], raw[:], cosb[:, js])
                t2 = tpool.tile([128, 512], F32, tag="t2")
                nc.vector.tensor_mul(t2[:], rot[:], sinb[:, js])
                dest = QTA[:, f, js] if f < HL else KT[:, f - HL, js]
                nc.vector.tensor_add(dest, t1[:], t2[:])

            # two-deep rolling prefetch of wqk feature groups
            wf_q = []
            pre_idx = [0]

            # Q0/Q1 first (the next window's front-pump reads head 0 right
            # away), K heads at slots 2-3: attention chunk w-1's diagonal
            # tiles (pumped at the start of window w) read chunk w-1's K,
            # which must not be the last thing window w-1 produces
            QK_ORDER = [0, 1, HL, HL + 1, 2, 3, 4, 5, 6, 7]

            def wf_prefetch(depth=2):
                while len(wf_q) < depth and pre_idx[0] < NJ * FQK:
                    wf_t = wpool.tile([128, ND, 128], BF, tag="wf")
                    nc.sync.dma_start(
                        out=wf_t[:], in_=wqk_d[QK_ORDER[pre_idx[0] % FQK]])
                    wf_q.append(wf_t)
                    pre_idx[0] += 1

            # only group 0 before the first x slices — a second 1MB weight
            # DMA here would delay x slice 0 past the warmup window
            wf_prefetch(depth=1)
            gen = None
            quotas = {0: 0, 1: 4, 2: 6, 3: 8}
            for w in range(NJ):
                if w > 0:
                    gen = attn_chunk(w - 1)
                quota = quotas[w]
                js = bass.ts(w, 512)
                xj = xpool.tile([128, ND, 512], BF)
                for n in range(ND):
                    nc.sync.dma_start(out=xj[:, n, :], in_=xt_d[:, n, js])
                # front-loaded pump covers the first x-slice arrivals
                pump(gen, 8)
                seq = ([("qk", f) for f in QK_ORDER]
                       + [("v", tt) for tt in range(4)])
                prev_rope = None
                for gi, (kind, f) in enumerate(seq):
                    if w == 0 and gi == 0:
                        # issued after the first x+w chunks so those DMAs lead
                        nc.sync.dma_start(out=cosb[:], in_=cos_d[:])
                        nc.sync.dma_start(out=sinb[:], in_=sin_d[:])
                    if w == 0 and gi == 5:
                        # 2MB; wv not needed until the V groups at slots 10-13
                        nc.sync.dma_start(out=wvb[:], in_=wv_d[:])
                    if kind == "qk":
                        wf = wf_q.pop(0)
                        wf_prefetch()
                        ps = ps_qk.tile([128, 512], F32, tag="qk")
                        for n in range(ND):
                            nc.tensor.matmul(ps[:], wf[:, n, :], xj[:, n, :],
                                             start=(n == 0), stop=(n == ND - 1))
                            # first group of a window is DMA-paced; attention
                            # tiles fill the slice waits
                            if gi == 0 and n % 4 == 3:
                                pump(gen, 1)
                        raw = tpool.tile([128, 512], BF, tag="raw")
                        nc.scalar.copy(out=raw[:], in_=ps[:])
                        pump(gen, quota)
                        # rope deferred one slot so its DMA/DVE never waits on
                        # the ACT copy while the PE has nothing else queued
                        if prev_rope is not None:
                            rope_emit(prev_rope)
                        prev_rope = (raw, f, js)
                    else:
                        tt = f
                        psv = ps_qk.tile([128, 512], F32, tag="qk")
                        for n in range(ND):
                            nc.tensor.matmul(psv[:, :KVL * HD],
                                             xj[:, n, bass.ts(tt, 128)],
                                             wvb[:, n, :],
                                             start=(n == 0), stop=(n == ND - 1))
                        nc.scalar.copy(out=V[:, w * 4 + tt, :],
                                       in_=psv[:, :KVL * HD])
                        pump(gen, quota)
                        if prev_rope is not None:
                            rope_emit(prev_rope)
                            prev_rope = None
                if prev_rope is not None:
                    rope_emit(prev_rope)
                    prev_rope = None
                pump(gen, 10 ** 6)  # exhaust chunk w-1 before next window

        # ---- window 4: attention chunk 3 + po filler, then po drain ----
        with tc.tile_pool(name="att_out", bufs=1) as att_out, \
             tc.tile_pool(name="spool", bufs=3) as spool, \
             tc.tile_pool(name="ps_po", bufs=2, space="PSUM") as ps_po:
            wob = att_out.tile([128, HL, D], BF)
            # nn-major slices: slice nn carries all 8 heads for output cols
            # [nn*512,(nn+1)*512), matching the nn-major pending_po order, so
            # the first po groups are runnable ~3us after window 4 opens
            for nn in range(D // 512):
                nc.sync.dma_start(out=wob[:, :, bass.ts(nn, 512)],
                                  in_=wo_d[:, :, bass.ts(nn, 512)])

            po_state = {"cur": None, "dd": 0}

            def po_step(budget):
                for _ in range(budget):
                    if po_state["cur"] is None:
                        if not pending_po:
                            return
                        qt, nn = pending_po.pop(0)
                        pop = ps_po.tile([128, 512], F32, tag="po")
                        po_state["cur"] = (qt, nn, pop)
                        po_state["dd"] = 0
                    qt, nn, pop = po_state["cur"]
                    dd = po_state["dd"]
                    nc.tensor.matmul(pop[:], QTA[:, dd, bass.ts(qt, 128)],
                                     wob[:, dd, bass.ts(nn, 512)],
                                     start=(dd == 0), stop=(dd == HL - 1))
                    po_state["dd"] += 1
                    if po_state["dd"] == HL:
                        stg = spool.tile([128, 512], BF, tag="stg")
                        nc.vector.tensor_copy(stg[:], pop[:])
                        nc.sync.dma_start(
                            out=po_d[bass.ts(qt, 128), bass.ts(nn, 512)],
                            in_=stg[:])
                        po_state["cur"] = None

            gen3 = attn_chunk(3)
            total_yields = 8 * (NT + 1) + 9
            cnt = 0
            while True:
                if next(gen3, _EXH) is _EXH:
                    break
                cnt += 1
                # pace po filler so it lasts until the generator is exhausted
                # (wob1 is already resident; wob2-dependent groups come last)
                rem = len(pending_po) * HL
                if po_state["cur"] is not None:
                    rem += HL - po_state["dd"]
                po_step(min(10, max(1, -(-rem // max(1, total_yields - cnt)))))
            while pending_po or po_state["cur"] is not None:
                po_step(8)

    nc.compile()
    return nc


def _build_legacy(mask_mode: str):
    """mask_mode: 'zero' | 'general' — non-causal fallback (v1 schedule)."""
    nc = bacc.Bacc("TRN2", target_bir_lowering=False, debug=False,
                   num_devices=N_CORES)

    xt_d = nc.dram_tensor("xt", [128, ND, S], BF, kind="ExternalInput").ap()
    wqk_d = nc.dram_tensor("wqk", [FQK, 128, ND, 128], BF, kind="ExternalInput").ap()
    wv_d = nc.dram_tensor("wv", [128, ND, KVL * HD], BF, kind="ExternalInput").ap()
    wo_d = nc.dram_tensor("wo", [128, HL, D], BF, kind="ExternalInput").ap()
    cos_d = nc.dram_tensor("cosd", [128, S], F32, kind="ExternalInput").ap()
    sin_d = nc.dram_tensor("sind", [128, S], F32, kind="ExternalInput").ap()
    pm_d = nc.dram_tensor("pm", [128, 128], BF, kind="ExternalInput").ap()
    if mask_mode == "general":
        mk_d = nc.dram_tensor("maskt", [S, S], BF, kind="ExternalInput").ap()
    po_d = nc.dram_tensor("po", [S, D], F32, kind="ExternalOutput").ap()

    def apply_tiles(j):
        if mask_mode == "general":
            return list(range(NT))
        return []

    with tile.TileContext(nc) as tc, ExitStack() as ctx:
        resident = ctx.enter_context(tc.tile_pool(name="resident", bufs=1))
        qkv = ctx.enter_context(tc.tile_pool(name="qkv", bufs=1))

        ones_col = resident.tile([128, 1], BF)
        nc.vector.memset(ones_col[:], 1.0)
        ones_row = resident.tile([1, 128], BF)
        nc.vector.memset(ones_row[:], 1.0)

        QT = qkv.tile([128, HL, S], BF)
        KT = qkv.tile([128, KVL, S], BF)
        V = qkv.tile([128, NT, KVL * HD], BF)

        with tc.tile_pool(name="s1const", bufs=1) as s1const, \
             tc.tile_pool(name="xpool", bufs=2) as xpool, \
             tc.tile_pool(name="wpool", bufs=2) as wpool, \
             tc.tile_pool(name="tpool", bufs=3) as tpool, \
             tc.tile_pool(name="ps_qk", bufs=2, space="PSUM") as ps_qk, \
             tc.tile_pool(name="ps_ro", bufs=2, space="PSUM") as ps_ro, \
             tc.tile_pool(name="ps_v", bufs=2, space="PSUM") as ps_v:
            pmb = s1const.tile([128, 128], BF)
            nc.sync.dma_start(out=pmb[:], in_=pm_d[:])
            cosb = s1const.tile([128, S], F32)
            sinb = s1const.tile([128, S], F32)
            wvb = s1const.tile([128, ND, KVL * HD], BF)
            for _ in range(128):
                wtile = ps_ro.tile([1, 128], F32, tag="warm")
                nc.tensor.matmul(wtile[:], ones_col[:], pmb[:], start=True, stop=True)

            def rope_emit(ent):
                raw, f, js = ent
                pq = ps_ro.tile([128, 512], F32, tag="ro")
                nc.tensor.matmul(pq[:], pmb[:], raw[:], start=True, stop=True)
                t1 = tpool.tile([128, 512], F32, tag="t1")
                nc.vector.tensor_mul(t1[:], raw[:], cosb[:, js])
                t2 = tpool.tile([128, 512], F32, tag="t2")
                nc.vector.tensor_mul(t2[:], pq[:], sinb[:, js])
                dest = QT[:, f, js] if f < HL else KT[:, f - HL, js]
                nc.vector.tensor_add(dest, t1[:], t2[:])

            wf_next = None
            for j in range(NJ):
                js = bass.ts(j, 512)
                if wf_next is None:
                    wf_next = wpool.tile([128, ND, 128], BF, tag="wf")
                    nc.sync.dma_start(out=wf_next[:], in_=wqk_d[0])
                xj = xpool.tile([128, ND, 512], BF)
                for n in range(ND):
                    nc.sync.dma_start(out=xj[:, n, :], in_=xt_d[:, n, js])
                for f in range(FQK):
                    wf = wf_next
                    nf = f + 1 if f + 1 < FQK else 0
                    if f + 1 < FQK or j + 1 < NJ:
                        wf_next = wpool.tile([128, ND, 128], BF, tag="wf")
                        nc.sync.dma_start(out=wf_next[:], in_=wqk_d[nf])
                    else:
                        wf_next = None
                    if j == 0 and f == 0:
                        nc.sync.dma_start(out=cosb[:], in_=cos_d[:])
                        nc.sync.dma_start(out=sinb[:], in_=sin_d[:])
                        nc.sync.dma_start(out=wvb[:], in_=wv_d[:])
                    ps = ps_qk.tile([128, 512], F32, tag="qk")
                    for n in range(ND):
                        nc.tensor.matmul(ps[:], wf[:, n, :], xj[:, n, :],
                                         start=(n == 0), stop=(n == ND - 1))
                    raw = tpool.tile([128, 512], BF, tag="raw")
                    nc.scalar.copy(out=raw[:], in_=ps[:])
                    rope_emit((raw, f, js))
                for tt in range(4):
                    psv = ps_v.tile([128, KVL * HD], F32, tag="v")
                    for n in range(ND):
                        nc.tensor.matmul(psv[:], xj[:, n, bass.ts(tt, 128)],
                                         wvb[:, n, :],
                                         start=(n == 0), stop=(n == ND - 1))
                    nc.scalar.copy(out=V[:, j * 4 + tt, :], in_=psv[:])

        att_out = ctx.enter_context(tc.tile_pool(name="att_out", bufs=1))
        attnT = att_out.tile([128, HL, S], BF)
        wob = att_out.tile([128, HL, D], BF)

        po_state = {"cur": None, "dd": 0}

        def po_step(budget):
            for _ in range(budget):
                if po_state["cur"] is None:
                    if not pending_po:
                        return
                    qt, nn = pending_po.pop(0)
                    pop = ps_po.tile([128, 512], F32, tag="po")
                    po_state["cur"] = (qt, nn, pop)
                    po_state["dd"] = 0
                qt, nn, pop = po_state["cur"]
                dd = po_state["dd"]
                nc.tensor.matmul(pop[:], attnT[:, dd, bass.ts(qt, 128)],
                                 wob[:, dd, bass.ts(nn, 512)],
                                 start=(dd == 0), stop=(dd == HL - 1))
                po_state["dd"] += 1
                if po_state["dd"] == HL:
                    stg = spool.tile([128, 512], F32, tag="stg")
                    nc.vector.tensor_copy(stg[:], pop[:])
                    nc.sync.dma_start(
                        out=po_d[bass.ts(qt, 128), bass.ts(nn, 512)], in_=stg[:])
                    po_state["cur"] = None

        with tc.tile_pool(name="mpool", bufs=1) as mpool, \
             tc.tile_pool(name="ppool", bufs=6) as ppool, \
             tc.tile_pool(name="qpool", bufs=2) as qpool, \
             tc.tile_pool(name="npool", bufs=4) as npool, \
             tc.tile_pool(name="spool", bufs=3) as spool, \
             tc.tile_pool(name="ps_st", bufs=2, space="PSUM") as ps_st, \
             tc.tile_pool(name="ps_o", bufs=2, space="PSUM") as ps_o, \
             tc.tile_pool(name="ps_l", bufs=1, space="PSUM") as ps_l, \
             tc.tile_pool(name="ps_b", bufs=1, space="PSUM") as ps_b, \
             tc.tile_pool(name="ps_po", bufs=2, space="PSUM") as ps_po:
            pending_po = []
            first_wo = True
            for j in range(NJ):
                js = bass.ts(j, 512)
                nkt = NT
                atiles = apply_tiles(j)
                if atiles:
                    msk = mpool.tile([128, len(atiles), 512], BF, tag="msk")
                    for idx, t in enumerate(atiles):
                        nc.sync.dma_start(out=msk[:, idx, :],
                                          in_=mk_d[bass.ts(t, 128), js])
                if first_wo:
                    for dd in range(HL):
                        nc.sync.dma_start(out=wob[:, dd, :], in_=wo_d[:, dd, :])
                    first_wo = False

                def emit_tail(ent):
                    th, tjs, trlh = ent
                    rbp = ps_b.tile([128, 512], F32, tag="rb")
                    nc.tensor.matmul(rbp[:], ones_row[:], trlh[:],
                                     start=True, stop=True)
                    nc.vector.tensor_mul(attnT[:, th, tjs], attnT[:, th, tjs],
                                         rbp[:])

                pending_tail = None
                for h in range(HL):
                    hk = h // (HL // KVL)
                    outp = ps_o.tile([128, 512], F32, tag="out")
                    lp = ps_l.tile([1, 512], F32, tag="l")
                    pts = []

                    def emit_pv(t):
                        nc.tensor.matmul(outp[:], V[:, t, bass.ts(hk, 128)],
                                         pts[t][:],
                                         start=(t == 0), stop=(t == nkt - 1))

                    for t in range(nkt):
                        if t == max(1, nkt // 2) and pending_tail is not None:
                            emit_tail(pending_tail)
                            pending_tail = None
                        stp = ps_st.tile([128, 512], F32, tag="st")
                        nc.tensor.matmul(stp[:], KT[:, hk, bass.ts(t, 128)],
                                         QT[:, h, js], start=True, stop=True)
                        pt = ppool.tile([128, 512], BF, tag="pt")
                        nc.scalar.activation(out=pt[:], in_=stp[:],
                                             func=mybir.ActivationFunctionType.Exp)
                        if t in atiles:
                            nc.vector.tensor_mul(
                                pt[:], pt[:], msk[:, atiles.index(t), :])
                        pts.append(pt)
                        po_step(2 if t % 2 else 1)
                        if t > 0:
                            emit_pv(t - 1)
                        if t % 4 == 3:
                            a, b, c, dq = pts[-4:]
                            s1 = qpool.tile([128, 512], BF, tag="s1")
                            nc.vector.tensor_add(s1[:], a[:], b[:])
                            s2 = qpool.tile([128, 512], BF, tag="s2")
                            nc.vector.tensor_add(s2[:], c[:], dq[:])
                            qd = qpool.tile([128, 512], BF, tag="qd")
                            nc.vector.tensor_add(qd[:], s1[:], s2[:])
                            nc.tensor.matmul(lp[:], ones_col[:], qd[:],
                                             start=(t == 3), stop=(t == nkt - 1))
                    emit_pv(nkt - 1)
                    nc.vector.tensor_copy(attnT[:, h, js], outp[:])
                    lnl = npool.tile([1, 512], F32, tag="lnl")
                    nc.scalar.activation(out=lnl[:], in_=lp[:],
                                         func=mybir.ActivationFunctionType.Ln)
                    rlh = npool.tile([1, 512], BF, tag="rlh")
                    nc.scalar.activation(out=rlh[:], in_=lnl[:],
                                         func=mybir.ActivationFunctionType.Exp,
                                         scale=-1.0)
                    pending_tail = (h, js, rlh)
                    po_step(16)
                if pending_tail is not None:
                    emit_tail(pending_tail)
                    pending_tail = None
                pending_po.extend(
                    (qt, nn) for qt in range(4 * j, 4 * j + 4)
                    for nn in range(D // 512))
            while pending_po or po_state["cur"] is not None:
                po_step(8)

    nc.compile()
    return nc


def _get_nc(mask_mode: str):
    if mask_mode not in _BUILD_CACHE:
        if mask_mode == "causal":
            _BUILD_CACHE[mask_mode] = _build_causal()
        else:
            _BUILD_CACHE[mask_mode] = _build_legacy(mask_mode)
    return _BUILD_CACHE[mask_mode]


_DEINT = np.concatenate([np.arange(0, HD, 2), np.arange(1, HD, 2)])  # de-interleave


def _host_prep(x, freqs_cos, freqs_sin, mask, wq, wk, wv, wo):
    bf16 = ml_dtypes.bfloat16
    scale = float(HD) ** -0.5

    # mask mode
    mask = np.asarray(mask, np.float32)
    tril = np.tril(np.ones((S, S), bool))
    if np.all(mask == 0):
        mask_mode = "zero"
    elif np.all(mask[tril] == 0) and np.all(mask[~tril] <= -1e8):
        mask_mode = "causal"
    else:
        mask_mode = "general"

    # weights: de-interleave head dims of wq/wk; fold softmax scale into wq
    wq_p = (np.asarray(wq, np.float32).reshape(H, HD, D)[:, _DEINT, :] * scale)
    wk_p = np.asarray(wk, np.float32).reshape(KVH, HD, D)[:, _DEINT, :]
    wv_n = np.asarray(wv, np.float32).reshape(KVH, HD, D)
    wo_n = np.asarray(wo, np.float32)

    per_group = []
    for g in range(GROUPS):
        feats = np.concatenate([
            wq_p[g * HL:(g + 1) * HL].reshape(HL * HD, D),
            wk_p[g * KVL:(g + 1) * KVL].reshape(KVL * HD, D),
        ], axis=0)  # [1280, D]
        wqk_dma = np.ascontiguousarray(
            feats.reshape(FQK, 128, ND, 128).transpose(0, 3, 2, 1)).astype(bf16)
        wvg = wv_n[g * KVL:(g + 1) * KVL].reshape(KVL * HD, D)
        wv_dma = np.ascontiguousarray(
            wvg.reshape(KVL * HD, ND, 128).transpose(2, 1, 0)).astype(bf16)
        woT = wo_n[:, g * HL * HD:(g + 1) * HL * HD].T  # [1024, D]
        wo_dma = np.ascontiguousarray(
            woT.reshape(HL, 128, D).transpose(1, 0, 2)).astype(bf16)
        per_group.append((wqk_dma, wv_dma, wo_dma))

    xs = []
    for b in range(B):
        xT = np.asarray(x[b], np.float32).T  # [D, S]
        xs.append(np.ascontiguousarray(
            xT.reshape(ND, 128, S).transpose(1, 0, 2)).astype(bf16))

    cosT = np.asarray(freqs_cos, np.float32).T  # [64, S]
    sinT = np.asarray(freqs_sin, np.float32).T
    cos_dma = np.ascontiguousarray(np.concatenate([cosT, cosT], 0))
    sin_dma = np.ascontiguousarray(np.concatenate([sinT, sinT], 0))
    if mask_mode == "causal":
        # rotate-half is a plain partition swap on device; the sign of the
        # rotated half lives here: rows 0-63 (the o_r lanes) get -sin
        cos_dma = cos_dma.astype(bf16)
        sin_dma = np.ascontiguousarray(
            np.concatenate([-sinT, sinT], 0)).astype(bf16)

    P = np.zeros((128, 128), np.float32)
    for r in range(64):
        P[r, 64 + r] = -1.0
        P[64 + r, r] = 1.0
    pm = np.ascontiguousarray(P.T).astype(bf16)

    mask_extra = {}
    if mask_mode == "causal":
        # ST[k,q] staircase band: valid iff q >= k within a 128x128 block
        mask_extra["trimask"] = np.triu(np.ones((128, 128), np.float32)).astype(bf16)
        sell = np.zeros((128, HL * 8), np.float32)
        selb = np.zeros((8, HL * 128), np.float32)
        for h in range(HL):
            sell[:, h * 8 + h] = 1.0
            selb[h, h * 128:(h + 1) * 128] = 1.0
        mask_extra["sell"] = sell.astype(bf16)
        mask_extra["selb"] = selb.astype(bf16)
    elif mask_mode == "general":
        with np.errstate(over="ignore"):
            mask_extra["maskt"] = np.ascontiguousarray(
                np.exp(mask.T)).astype(bf16)

    in_maps = []
    for c in range(N_CORES):
        b, g = c // GROUPS, c % GROUPS
        wqk_dma, wv_dma, wo_dma = per_group[g]
        m = {"xt": xs[b], "wqk": wqk_dma, "wv": wv_dma, "wo": wo_dma,
             "cosd": cos_dma, "sind": sin_dma}
        if mask_mode != "causal":
            m["pm"] = pm  # legacy path does RoPE via permutation matmul
        m.update(mask_extra)
        in_maps.append(m)
    return mask_mode, in_maps


def kernel(x, freqs_cos, freqs_sin, positions, mask, wq, wk, wv, wo,
           _want_profile=False):
    mask_mode, in_maps = _host_prep(x, freqs_cos, freqs_sin, mask, wq, wk, wv, wo)
    nc = _get_nc(mask_mode)
    res = run_bass_kernel_spmd(nc, in_maps, core_ids=list(range(N_CORES)),
                               trace=_want_profile)
    out = np.zeros((B, S, D), np.float32)
    for c in range(N_CORES):
        out[c // GROUPS] += res.results[c]["po"]
    if _want_profile:
        kernel.last_exec_time_ns = res.exec_time_ns
        kernel.last_results = res
    return out
